# revision 1
# baseline (speedup 1.0000x reference)
"""GraphSAGE (4-layer) forward pass on 8 Trainium2 NeuronCores — v2.

Improvements over v1:
  - Node layout quarter-aligned: table row = q*Q4 + core*SEG + lws, so the
    per-layer table AllGather splits into 4 chunked collectives (one per
    table quarter), each launched as soon as its source blocks of h_l are
    ready — collectives overlap edge processing of the current layer, and
    the next layer's first gathers start when chunk 0 lands.
  - Packed edge columns: per (group, quarter, dst-block) runs are padded to
    ceil(len/128) columns instead of a uniform worst-case T, cutting gather
    bytes / S-builds / matmuls by the padding slack.
  - Collective outputs are addr_space="Shared" (fast HBM-HBM path).
"""

import os
import sys
from dataclasses import dataclass

import numpy as np

for _p in ("/opt/trn_rl_repo", "/root/.axon_site/_ro/trn_rl_repo"):
    if os.path.isdir(_p) and _p not in sys.path:
        sys.path.append(_p)


def _bcast_ap(bass, t, mid, inner, expand_inner):
    """3D broadcast AP over a 2D [128, n] slice ``t``.

    expand_inner=True:  [p, c, j] -> t[p, c]   (ap [[part],[1,mid],[0,inner]])
    expand_inner=False: [p, c, j] -> t[p, j]   (ap [[part],[0,mid],[1,inner]])
    """
    if expand_inner:
        ap = [list(t.ap[0]), [1, mid], [0, inner]]
    else:
        ap = [list(t.ap[0]), [0, mid], [1, inner]]
    return bass.AP(t.tensor, t.offset, ap)

import ml_dtypes

BF16 = ml_dtypes.bfloat16


# --------------------------------------------------------------------------
# configuration
# --------------------------------------------------------------------------
@dataclass
class Cfg:
    gn: int  # nodes per graph
    gpc: list  # graphs per core (len 8)
    np_pad: int  # padded nodes per core (multiple of 512)
    dims: list  # [d0, d1, d2, d3, d4]
    grp: int = 8  # dst blocks per psum group
    ncores: int = 8

    @property
    def nb(self):  # 128-node blocks per core
        return self.np_pad // 128

    @property
    def seg(self):  # local nodes per table segment (quarter slice per core)
        assert self.np_pad % 512 == 0
        return self.np_pad // 4

    @property
    def q4(self):  # rows per table quarter
        q = self.ncores * self.seg
        assert q <= 32767, q
        return q

    @property
    def table_rows(self):
        return 4 * self.q4

    @property
    def g13(self):  # max graphs per core
        return max(self.gpc)

    @property
    def node_lo(self):
        lo = [0]
        for c in range(self.ncores):
            lo.append(lo[-1] + self.gpc[c] * self.gn)
        return lo

    @property
    def ngrp(self):
        assert self.nb % self.grp == 0
        return self.nb // self.grp


FULL_CFG = Cfg(
    gn=1000,
    gpc=[13, 13, 13, 13, 12, 12, 12, 12],
    np_pad=13312,
    dims=[128, 128, 118, 103, 5],
    grp=8,
)


# --------------------------------------------------------------------------
# host-side preprocessing (graph-structure sharding / packing)
# --------------------------------------------------------------------------
def preprocess(cfg: Cfg, src: np.ndarray, dst: np.ndarray):
    """Pack per-core edge streams with per-run ceil(len/128) columns.

    Stream layout per core: calls in (gi, q) order; within a call, blocks
    ascending, each block's run occupying ncol[call][ib]*128 slots.
    Returns dict with per-core arrays + the shared column structure
    (ncol identical across cores? no — per core) -> per-core ncol.
    """
    n = cfg.node_lo[-1]
    NB, GRP, NGRP, SEG = cfg.nb, cfg.grp, cfg.ngrp, cfg.seg
    src = np.asarray(src).astype(np.int64)
    dst = np.asarray(dst).astype(np.int64)
    deg = np.bincount(dst, minlength=n).astype(np.float64)
    invdeg = 1.0 / np.clip(deg, 1.0, None)

    lo = np.asarray(cfg.node_lo[:-1])
    core_of = np.searchsorted(np.asarray(cfg.node_lo[1:]), np.arange(n), side="right")
    local = np.arange(n) - lo[core_of]
    seg_of = local // SEG
    lidx_of = core_of * SEG + (local - seg_of * SEG)  # row within quarter

    # layer-4 collapse: per-graph pooled neighbor aggregation is linear in
    # h3, so per src node n and graph g: w[n, g] = sum over edges (n -> m),
    # m in g, of invdeg[m]. Each core holds w for ITS OWN nodes.
    ngraphs = sum(cfg.gpc)
    gid_of = np.arange(n) // cfg.gn  # graph of a (dst) node
    wflat = np.zeros(n * ngraphs, np.float64)
    np.add.at(wflat, src * ngraphs + gid_of[dst], invdeg[dst])
    w_all = wflat.reshape(n, ngraphs)

    ncalls = NGRP * 4
    out = dict(cores=[], w=[], ngraphs=ngraphs)
    for c in range(cfg.ncores):
        wc = np.zeros((cfg.np_pad, ngraphs), np.float64)
        wc[: cfg.node_lo[c + 1] - cfg.node_lo[c]] = w_all[
            cfg.node_lo[c] : cfg.node_lo[c + 1]
        ]
        # device layout [128, NB, ngraphs]: wpool[p, b, g] = wc[b*128+p, g]
        out["w"].append(
            np.ascontiguousarray(
                wc.reshape(NB, 128, ngraphs).transpose(1, 0, 2)
            ).astype(BF16)
        )
    for c in range(cfg.ncores):
        m = (dst >= cfg.node_lo[c]) & (dst < cfg.node_lo[c + 1])
        es, ed = src[m], dst[m]
        ld = ed - cfg.node_lo[c]
        b = ld // 128
        gi = b // GRP
        ib = b - gi * GRP
        q = seg_of[es]
        lidx = lidx_of[es]
        cid = gi * 4 + q
        order = np.lexsort((ld, ib, cid))
        es, ed, ld, b, gi, ib, q, lidx, cid = (
            x[order] for x in (es, ed, ld, b, gi, ib, q, lidx, cid)
        )
        # run = (cid, ib); counts
        rkey = cid * GRP + ib
        cnt = np.bincount(rkey, minlength=ncalls * GRP).reshape(ncalls, GRP)
        ncol = (cnt + 127) // 128  # [ncalls, GRP]
        call_cols = ncol.sum(axis=1)
        # bases in columns
        colbase_runs = np.concatenate([[0], np.cumsum(ncol.reshape(-1))])
        run_colbase = colbase_runs[:-1].reshape(ncalls, GRP)
        C = int(call_cols.sum())
        sl = C * 128
        # rank within run
        if len(rkey):
            kchange = np.r_[True, rkey[1:] != rkey[:-1]]
            run_start = np.maximum.accumulate(
                np.where(kchange, np.arange(len(rkey)), 0)
            )
            rank = np.arange(len(rkey)) - run_start
        else:
            rank = np.zeros(0, np.int64)
        pos = run_colbase.reshape(-1)[rkey] * 128 + rank
        assert pos.max(initial=0) < sl

        idx16 = np.zeros((16, sl // 16), np.int16)
        idx16[pos % 16, pos // 16] = lidx.astype(np.int16)
        idx128 = np.tile(idx16, (8, 1))

        sdst = np.full((128, C), -1.0, np.float32)
        sdst[pos % 128, pos // 128] = (ld - b * 128).astype(np.float32)
        ivd = np.zeros((128, C), np.float32)
        ivd[pos % 128, pos // 128] = invdeg[ed]
        gdst = np.full((128, C), -1.0, np.float32)
        gdst[pos % 128, pos // 128] = (ld // cfg.gn).astype(np.float32)

        out["cores"].append(
            dict(
                idx=idx128,
                sdst=sdst,
                ivd=ivd,
                gdst=gdst,
                ncol=ncol,
                C=C,
                sl=sl,
            )
        )
    return out


def pack_weights(cfg: Cfg, inp: dict):
    d = cfg.dims
    w = {}
    for l in range(1, 4):
        din, dout = d[l - 1], d[l]
        wn = np.zeros((128, 128), np.float32)
        ws = np.zeros((128, 128), np.float32)
        wn[:din, :dout] = np.asarray(inp[f"wn{l}"], np.float32)
        ws[:din, :dout] = np.asarray(inp[f"ws{l}"], np.float32)
        bb = np.zeros((128, 1), np.float32)
        bb[:dout, 0] = np.asarray(inp[f"b{l}"], np.float32)
        w[f"wn{l}"] = wn.astype(BF16)
        w[f"ws{l}"] = ws.astype(BF16)
        w[f"b{l}"] = bb
    din, dout = d[3], d[4]
    wn4 = np.zeros((128, 8), np.float32)
    ws4 = np.zeros((128, 8), np.float32)
    wn4[:din, :dout] = np.asarray(inp["wn4"], np.float32)
    ws4[:din, :dout] = np.asarray(inp["ws4"], np.float32)
    b4r = np.zeros((1, 8), np.float32)
    b4r[0, :dout] = np.asarray(inp["b4"], np.float32) * float(cfg.gn)
    w["wn4"] = wn4
    w["ws4"] = ws4
    w["b4r"] = b4r
    return w


def shard_infeat(cfg: Cfg, in_feat: np.ndarray):
    d0 = cfg.dims[0]
    shards = []
    for c in range(cfg.ncores):
        lo, hi = cfg.node_lo[c], cfg.node_lo[c + 1]
        h = np.zeros((128, cfg.np_pad), np.float32)
        h[:d0, : hi - lo] = np.asarray(in_feat[lo:hi], np.float32).T
        shards.append(h.astype(BF16))
    return shards


# --------------------------------------------------------------------------
# device program
# --------------------------------------------------------------------------
def build_nc(cfg: Cfg, ncol: np.ndarray, ngraphs: int = 100,
             no_collective: bool = False):
    """ncol: [NGRP*4, GRP] int array — columns per (call, block-in-group).

    NOTE: ncol must be identical across cores (SPMD single program). The
    driver pads per-core ncol to the elementwise max.
    """
    from concourse import bacc, bass, tile, mybir

    dt = mybir.dt
    d = cfg.dims
    NB, GRP, NGRP = cfg.nb, cfg.grp, cfg.ngrp
    NP, SEG, Q4, GN, G13 = cfg.np_pad, cfg.seg, cfg.q4, cfg.gn, cfg.g13
    SEGB = SEG // 128  # blocks per segment

    ncol = np.asarray(ncol)
    ncalls = NGRP * 4
    assert ncol.shape == (ncalls, GRP)
    call_cols = ncol.sum(axis=1)
    call_base = np.concatenate([[0], np.cumsum(call_cols)])  # in columns
    C = int(call_cols.sum())
    sl = C * 128
    MAXC = int(call_cols.max())

    NQ = int(os.environ.get("GATHER_QUEUES2", "4"))  # concurrent SWDGE queues
    _DBG2 = os.environ.get("DBG2", "full")
    nc = bacc.Bacc(
        "TRN2",
        target_bir_lowering=False,
        debug=False,
        num_devices=cfg.ncores,
        num_swdge_queues=NQ,
        dynamic_dma_scratch_size=int(os.environ.get("DMA_SCRATCH2", "16384")),
    )

    # ---- I/O -------------------------------------------------------------
    h0t_d = nc.dram_tensor("h0t", [128, NP], dt.bfloat16, kind="ExternalInput")
    idx_d = nc.dram_tensor("idx", [128, sl // 16], dt.int16, kind="ExternalInput")
    sdst_d = nc.dram_tensor("sdst", [128, C], dt.bfloat16, kind="ExternalInput")
    ivd_d = nc.dram_tensor("ivd", [128, C], dt.bfloat16, kind="ExternalInput")
    wpool_d = nc.dram_tensor(
        "wpool", [128, NB * ngraphs], dt.bfloat16, kind="ExternalInput"
    )
    sel_d = nc.dram_tensor("sel", [ngraphs, 16], dt.float32, kind="ExternalInput")
    wd = {}
    for l in range(1, 4):
        wd[f"wn{l}"] = nc.dram_tensor(f"wn{l}", [128, 128], dt.bfloat16, kind="ExternalInput")
        wd[f"ws{l}"] = nc.dram_tensor(f"ws{l}", [128, 128], dt.bfloat16, kind="ExternalInput")
        wd[f"b{l}"] = nc.dram_tensor(f"b{l}", [128, 1], dt.float32, kind="ExternalInput")
    wd["wn4"] = nc.dram_tensor("wn4", [128, 8], dt.float32, kind="ExternalInput")
    wd["ws4"] = nc.dram_tensor("ws4", [128, 8], dt.float32, kind="ExternalInput")
    wd["b4r"] = nc.dram_tensor("b4r", [1, 8], dt.float32, kind="ExternalInput")
    out_d = nc.dram_tensor("out", [G13, 8], dt.float32, kind="ExternalOutput")

    with tile.TileContext(nc) as tc:
        with (
            tc.tile_pool(name="resident", bufs=1) as rp,
            tc.tile_pool(name="dram", bufs=1, space="DRAM") as dp,
            tc.tile_pool(name="gather", bufs=3) as gp,
            tc.tile_pool(name="spool", bufs=2) as sp,
            tc.tile_pool(name="stage", bufs=3) as stp,
            tc.tile_pool(name="psum_agg", bufs=4, space="PSUM") as pag,
            tc.tile_pool(name="psum_misc", bufs=2, space="PSUM") as pms,
            tc.tile_pool(name="hts", bufs=2) as hp,
        ):
            # ---- resident SBUF tensors ----------------------------------
            idx_s = rp.tile([128, sl // 16], dt.int16)
            sdst_s = rp.tile([128, C], dt.bfloat16)
            ivd_s = rp.tile([128, C], dt.bfloat16)
            wpool_s = rp.tile([128, NB * ngraphs], dt.bfloat16)
            sel_s = rp.tile([ngraphs, 16], dt.float32)
            nc.sync.dma_start(out=wpool_s[:], in_=wpool_d[:])
            nc.sync.dma_start(out=sel_s[:], in_=sel_d[:])
            ws_s = {}
            for k, dd in wd.items():
                if k in ("wn4", "ws4"):
                    t = rp.tile([128, 8], dt.float32, name=f"w_{k}")
                elif k == "b4r":
                    t = rp.tile([1, 8], dt.float32, name=f"w_{k}")
                elif k.startswith("b"):
                    t = rp.tile([128, 1], dt.float32, name=f"w_{k}")
                else:
                    t = rp.tile([128, 128], dt.bfloat16, name=f"w_{k}")
                ws_s[k] = t
                nc.sync.dma_start(out=t[:], in_=dd[:])
            nc.sync.dma_start(out=idx_s[:], in_=idx_d[:])
            nc.sync.dma_start(out=sdst_s[:], in_=sdst_d[:])
            nc.sync.dma_start(out=ivd_s[:], in_=ivd_d[:])

            # constants
            iota_i = rp.tile([128, 128], dt.int32)
            nc.gpsimd.iota(iota_i[:], pattern=[[1, 128]], base=0, channel_multiplier=0)
            iota_b = rp.tile([128, 128], dt.bfloat16)
            nc.vector.tensor_copy(iota_b[:], iota_i[:])
            pidx_i = rp.tile([128, 1], dt.int32)
            nc.gpsimd.iota(pidx_i[:], pattern=[[1, 1]], base=0, channel_multiplier=1)
            pidx_f = rp.tile([128, 1], dt.float32)
            nc.vector.tensor_copy(pidx_f[:], pidx_i[:])
            ident_b = rp.tile([128, 128], dt.bfloat16)
            nc.vector.tensor_scalar(
                ident_b[:], iota_b[:], pidx_f[:], None, mybir.AluOpType.is_equal
            )
            ones_row = rp.tile([1, G13], dt.float32)
            nc.vector.memset(ones_row[:], 1.0)

            # h tiles (transposed feature-major, bf16)
            ht = [None] * 4
            ht[0] = hp.tile([128, NP], dt.bfloat16, tag="ht", name="ht0")
            nc.sync.dma_start(out=ht[0][:], in_=h0t_d[:])

            # DRAM tables: per layer (1..3), per quarter, Shared for
            # collectives (a Shared tile allows only a single writer inst)
            tables = [
                [
                    dp.tile([Q4, 128], dt.bfloat16, name=f"table{s}_{q}",
                            addr_space="Local" if no_collective else "Shared")
                    for q in range(4)
                ]
                for s in range(3)
            ]
            agins = [
                [
                    dp.tile([SEG, 128], dt.bfloat16, name=f"agin{s}_{q}")
                    for q in range(4)
                ]
                for s in range(3)
            ]
            # layer-4 pooled-agg AllReduce buffers
            arin = dp.tile([128, ngraphs], dt.float32, name="arin")
            arout = dp.tile([128, ngraphs], dt.float32, name="arout",
                            addr_space="Local" if no_collective else "Shared")

            def project_seg(l, src_ht, slot, s):
                """Project segment s of layer-l table; launch its collective."""
                din = d[l - 1]
                agin = agins[slot][s]
                for j in range(SEGB):
                    b = s * SEGB + j
                    st = stp.tile([128, 128], dt.bfloat16, tag="stage")
                    pp = pms.tile([128, 128], dt.float32, tag="proj", bufs=1)
                    nc.tensor.matmul(
                        pp[:, :],
                        src_ht[:din, b * 128 : (b + 1) * 128],
                        ws_s[f"wn{l}"][:din, :],
                        start=True,
                        stop=True,
                    )
                    nc.scalar.copy(st[:, :], pp[:, :])
                    nc.sync.dma_start(
                        out=agin[j * 128 : (j + 1) * 128, :],
                        in_=st[:, :],
                    )
                if no_collective:
                    for r in range(cfg.ncores):
                        nc.sync.dma_start(
                            out=tables[slot][s][r * SEG : (r + 1) * SEG, :],
                            in_=agin[:, :],
                        )
                else:
                    nc.gpsimd.collective_compute(
                        "AllGather",
                        mybir.AluOpType.bypass,
                        replica_groups=[list(range(cfg.ncores))],
                        ins=[agin.opt()],
                        outs=[tables[slot][s].opt()],
                    )

            # per-bank last-touch bookkeeping: for layer l (l<4), bank j of
            # group gi: last (q, ib, t) edge matmul, else self matmul.
            def bank_last(gi):
                """For each bank j in group gi -> ('edge', q, ib, t) or None."""
                res = {}
                for j in range((GRP + 3) // 4):
                    last = None
                    for q in range(4):
                        for ib in range(j * 4, min(j * 4 + 4, GRP)):
                            nc_ = ncol[gi * 4 + q, ib]
                            if nc_ > 0:
                                last = (q, ib, int(nc_) - 1)
                    res[j] = last
                return res

            def edge_phase(l, slot, src_ht, on_group_done):
                din = d[l - 1]
                dout = d[l]
                tq = tables[slot]
                for gi in range(NGRP):
                    nbank = (GRP + 3) // 4
                    banks = [
                        pag.tile(
                            [128, 512], dt.float32, tag="aggbank",
                            name=f"aggbank_{l}_{gi}_{jj}",
                        )
                        for jj in range(nbank)
                    ]
                    lasts = bank_last(gi)
                    # self path first: hsT = ws^T . h
                    for ib in range(GRP):
                        b = gi * GRP + ib
                        ptile = banks[ib // 4]
                        pslice = ptile[:, (ib % 4) * 128 : (ib % 4 + 1) * 128]
                        nc.tensor.matmul(
                            pslice[:, :],
                            ws_s[f"ws{l}"][:din, :],
                            src_ht[:din, b * 128 : (b + 1) * 128],
                            start=(ib % 4 == 0),
                            stop=(lasts[ib // 4] is None and ib % 4 == 3),
                        )
                    for q in range(4):
                        cid = gi * 4 + q
                        ccols = int(call_cols[cid])
                        if ccols == 0:
                            continue
                        gt = gp.tile([128, MAXC, 128], dt.bfloat16, tag="gather")
                        e0 = int(call_base[cid]) * 128
                        nidx = ccols * 128
                        if _DBG2 == "nogather":
                            nc.vector.memset(gt[:, :ccols, :], 0.0)
                        else:
                            # split the call across NQ SWDGE queues so the
                            # SDMA engines overlap multiple descriptor
                            # streams (~NQ x the serial gather throughput)
                            percol = (ccols + NQ - 1) // NQ
                            ci = 0
                            j = 0
                            while ci < nidx:
                                ilen = min(percol * 128, nidx - ci)
                                nc.gpsimd.dma_gather(
                                    gt[:, ci // 128 : (ci + ilen) // 128, :],
                                    tq[q][:, :],
                                    idx_s[
                                        :, (e0 + ci) // 16 : (e0 + ci + ilen) // 16
                                    ],
                                    ilen,
                                    ilen,
                                    128,
                                    elem_step=128,
                                    single_packet=False,
                                    queue_num=j % NQ,
                                )
                                ci += ilen
                                j += 1
                        # call-wide S build: 2 DVE passes with broadcast APs
                        b0 = int(call_base[cid])
                        key2 = sdst_s[:, b0 : b0 + ccols]
                        ivd2 = ivd_s[:, b0 : b0 + ccols]
                        sbig = sp.tile([128, MAXC, 128], dt.bfloat16, tag="S")
                        nc.vector.scalar_tensor_tensor(
                            sbig[:, :ccols, :],
                            _bcast_ap(bass, iota_b[:, :], ccols, 128, False),
                            1.0,
                            _bcast_ap(bass, key2, ccols, 128, True),
                            mybir.AluOpType.mult,
                            mybir.AluOpType.is_equal,
                        )
                        nc.vector.scalar_tensor_tensor(
                            sbig[:, :ccols, :],
                            sbig[:, :ccols, :],
                            1.0,
                            _bcast_ap(bass, ivd2, ccols, 128, True),
                            mybir.AluOpType.mult,
                            mybir.AluOpType.mult,
                        )
                        col = 0
                        for ib in range(GRP):
                            ncb = int(ncol[cid, ib])
                            for t in range(ncb):
                                ptile = banks[ib // 4]
                                pslice = ptile[
                                    :, (ib % 4) * 128 : (ib % 4 + 1) * 128
                                ]
                                last = lasts[ib // 4] == (q, ib, t)
                                nc.tensor.matmul(
                                    pslice[:, :],
                                    gt[:, col, :],
                                    sbig[:, col, :],
                                    start=False,
                                    stop=last,
                                )
                                col += 1
                    # epilogue per bank: h_l = relu(psum + b)
                    for j, ptile in enumerate(banks):
                        w = min(512, (GRP - j * 4) * 128)
                        c0 = (gi * GRP + j * 4) * 128
                        nc.scalar.activation(
                            ht[l][:dout, c0 : c0 + w],
                            ptile[:dout, :w],
                            mybir.ActivationFunctionType.Relu,
                            bias=ws_s[f"b{l}"][:dout, 0:1],
                        )
                    on_group_done(gi)

            # ---------------- main schedule ------------------------------
            # segment s of a layer's table needs blocks [s*SEGB, (s+1)*SEGB)
            # -> ready after group floor(((s+1)*SEGB - 1)/GRP) of the
            # producing edge phase.
            def seg_ready_group(s):
                return ((s + 1) * SEGB - 1) // GRP

            def make_cb(l_next, slot_next):
                """Project+collective chunks of layer l_next's table as the
                producing layer's groups complete."""
                ready = {seg_ready_group(s): s for s in range(4)}

                def cb(gi):
                    if gi in ready:
                        project_seg(l_next, ht[l_next - 1], slot_next, ready[gi])

                return cb

            def _sched():
                d3 = d[3]
                # layer 1 table from ht[0] (input): project all segments now
                for s in range(4):
                    project_seg(1, ht[0], 0, s)
                ht[1] = hp.tile([128, NP], dt.bfloat16, tag="ht", name="ht1")
                edge_phase(1, 0, ht[0], make_cb(2, 1))

                ht[2] = hp.tile([128, NP], dt.bfloat16, tag="ht", name="ht2")
                edge_phase(2, 1, ht[1], make_cb(3, 2))

                # layer-4 pooled aggregation is linear in h3 with fixed
                # host-precomputed weights w[n, g]: emit per-block
                # transpose(h3) @ w partial matmuls as layer-3 groups finish.
                ppagg = pms.tile([128, ngraphs], dt.float32, tag="l4agg",
                                 bufs=1)

                def l4cb(gi):
                    for ib in range(GRP):
                        b = gi * GRP + ib
                        ppb = pms.tile([128, 128], dt.bfloat16, tag="projT",
                                       bufs=1)
                        nc.tensor.transpose(
                            ppb[:, :d3],
                            ht[3][:d3, b * 128 : (b + 1) * 128],
                            ident_b[:d3, :d3],
                        )
                        st = stp.tile([128, 128], dt.bfloat16, tag="stage")
                        nc.scalar.copy(st[:, :d3], ppb[:, :d3])
                        nc.tensor.matmul(
                            ppagg[:, :],
                            st[:, :],
                            wpool_s[:, b * ngraphs : (b + 1) * ngraphs],
                            start=(b == 0),
                            stop=(b == NB - 1),
                        )

                ht[3] = hp.tile([128, NP], dt.bfloat16, tag="ht", name="ht3")
                edge_phase(3, 2, ht[2], l4cb)

                # AllReduce the pooled aggregation across cores (tiny)
                arbuf = rp.tile([128, ngraphs], dt.float32)
                nc.scalar.copy(arbuf[:, :], ppagg[:, :])
                nc.sync.dma_start(out=arin[:, :], in_=arbuf[:, :])
                if no_collective:
                    nc.sync.dma_start(out=arout[:, :], in_=arin[:, :])
                else:
                    nc.gpsimd.collective_compute(
                        "AllReduce",
                        mybir.AluOpType.add,
                        replica_groups=[list(range(cfg.ncores))],
                        ins=[arin.opt()],
                        outs=[arout.opt()],
                    )
                pagg_s = rp.tile([128, ngraphs], dt.float32)
                nc.sync.dma_start(out=pagg_s[:, :], in_=arout[:, :])

                # pf100[g, o] = sum_f pagg_s[f, g] * wn4[f, o]
                pf100 = pag.tile([ngraphs, 8], dt.float32, tag="aggbank")
                nc.tensor.matmul(
                    pf100[:, : d[4]],
                    pagg_s[:d3, :ngraphs],
                    ws_s["wn4"][:d3, : d[4]],
                    start=True,
                    stop=True,
                )
                pf100_s = rp.tile([ngraphs, 8], dt.float32)
                nc.scalar.copy(pf100_s[:, :], pf100[:, :])

                # pooled_h3T[f, g] = sum over graph g's node columns of h3T
                ph3 = rp.tile([128, G13], dt.float32)
                for g in range(G13):
                    nc.vector.tensor_reduce(
                        ph3[:d3, g : g + 1],
                        ht[3][:d3, g * GN : (g + 1) * GN],
                        mybir.AxisListType.X,
                        mybir.AluOpType.add,
                    )

                pf = pms.tile([G13, 8], dt.float32, tag="small4", bufs=1)
                nc.tensor.matmul(
                    pf[:, : d[4]], ph3[:d3, :G13], ws_s["ws4"][:d3, : d[4]],
                    start=True, stop=False,
                )
                nc.tensor.matmul(
                    pf[:, : d[4]],
                    sel_s[:ngraphs, :G13],
                    pf100_s[:ngraphs, : d[4]],
                    start=False, stop=False,
                )
                nc.tensor.matmul(
                    pf[:, : d[4]], ones_row[0:1, :G13], ws_s["b4r"][0:1, : d[4]],
                    start=False, stop=True,
                )
                outs = rp.tile([G13, 8], dt.float32)
                nc.vector.tensor_scalar(
                    outs[:, : d[4]], pf[:, : d[4]], 1.0 / GN, None,
                    mybir.AluOpType.mult,
                )
                nc.sync.dma_start(out=out_d[:, : d[4]], in_=outs[:, : d[4]])

            _sched()

    nc.compile()
    return nc


# --------------------------------------------------------------------------
# driver
# --------------------------------------------------------------------------
def make_in_maps(cfg: Cfg, inputs: dict):
    prep = preprocess(cfg, inputs["src"], inputs["dst"])
    # SPMD: one program for all cores -> pad ncol to elementwise max, and
    # repack each core's stream into the shared column layout.
    ncol = np.maximum.reduce([pc["ncol"] for pc in prep["cores"]])
    call_cols = ncol.sum(axis=1)
    C = int(call_cols.sum())
    sl = C * 128

    w = pack_weights(cfg, inputs)
    shards = shard_infeat(cfg, inputs["in_feat"])
    in_maps = []
    for c in range(cfg.ncores):
        pc = prep["cores"][c]
        # remap this core's columns into the padded shared layout
        src_base = np.concatenate([[0], np.cumsum(pc["ncol"].reshape(-1))])
        dst_runbase = np.concatenate([[0], np.cumsum(ncol.reshape(-1))])
        idx = np.zeros((128, sl // 16), np.int16)
        sdst = np.full((128, C), -1.0, np.float32)
        ivd = np.zeros((128, C), np.float32)
        gdst = np.full((128, C), -1.0, np.float32)
        nruns = ncol.size
        for r in range(nruns):
            w_src = int(pc["ncol"].reshape(-1)[r])
            if w_src == 0:
                continue
            s0, d0 = int(src_base[r]), int(dst_runbase[r])
            sdst[:, d0 : d0 + w_src] = pc["sdst"][:, s0 : s0 + w_src]
            ivd[:, d0 : d0 + w_src] = pc["ivd"][:, s0 : s0 + w_src]
            gdst[:, d0 : d0 + w_src] = pc["gdst"][:, s0 : s0 + w_src]
            idx[:, d0 * 8 : (d0 + w_src) * 8] = pc["idx"][:, s0 * 8 : (s0 + w_src) * 8]
        ngraphs = prep["ngraphs"]
        g0 = sum(cfg.gpc[:c])
        sel = np.zeros((ngraphs, 16), np.float32)
        for j in range(cfg.gpc[c]):
            sel[g0 + j, j] = 1.0
        m = dict(
            h0t=shards[c],
            idx=idx,
            sdst=sdst.astype(BF16),
            ivd=ivd.astype(BF16),
            wpool=prep["w"][c].reshape(128, -1),
            sel=sel,
        )
        m.update(w)
        in_maps.append(m)
    return dict(ncol=ncol, C=C, sl=sl, ngraphs=prep["ngraphs"]), in_maps


def assemble_output(cfg: Cfg, results):
    ngraphs = sum(cfg.gpc)
    out = np.zeros((ngraphs, cfg.dims[4]), np.float32)
    g0 = 0
    for c in range(cfg.ncores):
        r = results[c]["out"]
        out[g0 : g0 + cfg.gpc[c]] = np.asarray(r, np.float32)[: cfg.gpc[c], : cfg.dims[4]]
        g0 += cfg.gpc[c]
    return out


_CACHE = {}


def kernel(**inputs) -> np.ndarray:
    cfg = FULL_CFG
    prep, in_maps = make_in_maps(cfg, inputs)
    key = ("nc2", prep["ncol"].tobytes())
    if key not in _CACHE:
        _CACHE[key] = build_nc(cfg, prep["ncol"], prep["ngraphs"])
    nc = _CACHE[key]
    from concourse.bass_utils import run_bass_kernel_spmd

    res = run_bass_kernel_spmd(nc, in_maps, core_ids=list(range(cfg.ncores)))
    return assemble_output(cfg, res.results)

